# revision 29
# baseline (speedup 1.0000x reference)
"""BasePNARetriever Trainium2 kernel (8 NeuronCores, SPMD).

Strategy (v6, ~2.1x over the 946us v2 baseline):
  - A tiny AllGather at t~0 acts as a cross-core barrier so the real
    AllGather's peer-skew wait (~27us measured) mostly disappears.
  - Phase A (vocab-sharded down-projection) in bf16: each core streams a
    [4096, 4096] bf16 slice of text_embeddings via HWDGE (nc.sync - NOT
    SWDGE, so emb loads never queue behind gather desc-gen on GpSimd) and
    computes RtabT[64, 4096] on PE (bf16 matmul, fp32 PSUM accumulate).
    PE-transposes back to row-major; ACT emits the bf16 table slice with
    rows packed [val(64) | val^2(64)] (256B rows: the squares ride the
    gather for free). One AllGather builds rtab2[32768, 128] bf16 in DRAM.
  - Phase C: gather descriptor generation is the wall. Measured on HW:
    7.9 ns/descriptor per Q7 pair, but instructions on DIFFERENT SWDGE
    queues overlap across the four Q7 pairs (~2 ns/desc effective), so
    the 25 chunk-gathers round-robin queues 0-3. The gathers follow the
    AllGather (RAW on rtab2) and desc-gen runs concurrently with pooling.
  - Pooling per 2-tile chunk, engine-partitioned to avoid the DVE-2-port
    vs Q7-descriptor-ring SBUF lock: DVE runs ONLY tensor_tensor trees
    (never contends): sum via 4 bf16 halvings, max/min via 4 bf16
    halvings each; ACT (never contends) does mean/sq-mean scaling with
    per-row activation(scale=...), squares, std sqrt(+eps bias), the
    G1/G2 scale combines, and Square+accum_out for the L2 row norms; PE
    does the FC via PE-transposed bf16 features.
  - Host precomputes lengths/log-scales and patches the rare rows
    containing id==0 tokens (~25 rows in 50000).
  - KPREP=1 (experimental, off by default) additionally pre-generates
    gather descriptors during phase A via prepare_only + trigger_dma;
    it overlaps desc-gen fully (-70us) but currently returns corrupted
    data - the prep-DMA completion semaphores are not correctly threaded
    to the pooling readers.
"""

import sys

sys.path.insert(0, "/opt/trn_rl_repo")

import os

import numpy as np

import concourse.bass as bass
import concourse.bacc as bacc
import concourse.mybir as mybir
import concourse.tile as tile
from concourse.bass_utils import run_bass_kernel_spmd

F32 = mybir.dt.float32
BF = mybir.dt.bfloat16
I16 = mybir.dt.int16
AF = mybir.ActivationFunctionType
ALU = mybir.AluOpType

NCORES = 8
VOCAB, HID, R, B, S = 32000, 4096, 64, 50000, 16
VSH = VOCAB // NCORES          # 4000 real vocab rows per core
VSHP = 4096                    # padded vocab rows per core (32 x 128)
VOCABP = VSHP * NCORES         # 32768 padded vocab
KC = HID // 128                # 32 contraction chunks
BSH = B // NCORES              # 6250 rows per core
NT = 49                        # row tiles of 128 (6272 padded rows)
BPAD = NT * 128                # 6272
E2 = 2 * R                     # 128: packed table row [val(64)|sq(64)]
CH_T = 2                       # row-tiles per gather chunk
# first 4 chunks are single-tile so the first gather lands (and DVE starts)
# earlier; the rest are 2-tile
CHUNKS = [(t, 1) for t in range(4)] + [
    (i, min(CH_T, NT - i)) for i in range(4, NT, CH_T)
]
NQ = 4                         # SWDGE queues
_GATE_CTR = [0]


def _GATE_SEQ():
    _GATE_CTR[0] += 1
    return _GATE_CTR[0]

AG_AT = 8                      # emit the AllGather after this many preps
TRIG_AT = 12                   # first triggers after this many preps


def _phase_c(nc, tc, psT, rtab2, idx_sb, aux_sb, wret_sb, biasr_sb, identb_sb,
             ostage, eps_sb, emit_ag, out_ap):
    stage = os.environ.get("KSTAGE", "full")
    prep_mode = os.environ.get("KPREP", "0") == "1"
    nbuf = int(os.environ.get("KBUFS", "11"))
    with (
        tc.tile_pool(name="g", bufs=nbuf) as gpool,
        tc.tile_pool(name="tr", bufs=2) as tpool,
        tc.tile_pool(name="f", bufs=2) as fpool,
        tc.tile_pool(name="psG", bufs=2, space="PSUM") as psG,
    ):
        psF = psT  # reuse the open pool: bank history already PE-observed
        dma_sems = (
            [nc.alloc_semaphore(f"gsem{q}") for q in range(NQ)] if prep_mode else None
        )
        gtiles = {}
        state = {"pending": None}

        def emit_prep(ci):
            t0, ntile = CHUNKS[ci]
            nidx = ntile * 2048
            nslot = ntile * 16
            g = gpool.tile([128, CH_T * 16, E2], BF, tag="g")
            q = ci % NQ
            if stage in ("gather", "full"):
                kw = {}
                if prep_mode:
                    kw = dict(prepare_only=True, sem=dma_sems[q])
                nc.gpsimd.dma_gather(
                    g[:, :nslot, :],
                    rtab2[:],
                    idx_sb[:, t0 * 128 : t0 * 128 + nidx // 16],
                    nidx,
                    nidx,
                    E2,
                    single_packet=False,
                    queue_num=q,
                    **kw,
                )
            gtiles[ci] = g

        def emit_trigger(ci):
            if prep_mode and stage in ("gather", "full"):
                nc.gpsimd.trigger_dma(count=None, queue_num=ci % NQ)

        def finish(t0, ntile, gps):
            # res = G0 + G1*scale + G2*iscale + bias, then L2 normalize.
            # Deferred one chunk so the DVE queue never stalls on the PE/ACT
            # round-trip that produces gps.
            res = fpool.tile([128, CH_T, R], F32, tag="res")
            r2 = fpool.tile([128, CH_T, R], F32, tag="r2")
            for tt in range(ntile):
                t = t0 + tt
                nc.scalar.activation(
                    res[:, tt, :], gps[:, tt, R : 2 * R], AF.Copy,
                    scale=aux_sb[:, NT + t : NT + t + 1])
                nc.scalar.activation(
                    r2[:, tt, :], gps[:, tt, 2 * R : 3 * R], AF.Copy,
                    scale=aux_sb[:, 2 * NT + t : 2 * NT + t + 1])
            nc.vector.tensor_tensor(
                res[:, :ntile], res[:, :ntile], gps[:, :ntile, 0:R], ALU.add)
            nc.vector.tensor_add(res[:, :ntile], res[:, :ntile], r2[:, :ntile])
            nc.vector.tensor_tensor(
                ostage[:, t0 : t0 + ntile, :], res[:, :ntile],
                biasr_sb[:, None, :].broadcast_to([128, ntile, R]), ALU.add)
            # L2 norm: ACT Square + accum_out gives the row sum of squares
            sqr = fpool.tile([128, CH_T, R], F32, tag="sqr")
            ss = fpool.tile([128, CH_T], F32, tag="ss")
            for tt in range(ntile):
                nc.scalar.activation(
                    sqr[:, tt, :], ostage[:, t0 + tt, :], AF.Square,
                    accum_out=ss[:, tt : tt + 1])
            iss = fpool.tile([128, CH_T], F32, tag="iss")
            nc.vector.reciprocal(iss[:, :ntile], ss[:, :ntile])
            rin = fpool.tile([128, CH_T], F32, tag="rin")
            nc.scalar.sqrt(rin[:, :ntile], iss[:, :ntile])
            nc.vector.tensor_mul(
                ostage[:, t0 : t0 + ntile, :], ostage[:, t0 : t0 + ntile, :],
                rin[:, :ntile][:, :, None].broadcast_to([128, ntile, R]))

        def emit_pool(ci):
            if stage != "full":
                return
            t0, ntile = CHUNKS[ci]
            g = gtiles.pop(ci)
            nslot = ntile * 16
            g4 = g[:, :nslot, :].rearrange("p (t s) e -> p t s e", s=16)

            if prep_mode:
                # all direct readers of g are DVE ops; gate the DVE stream on
                # the chunk's DMA completion (per-queue FIFO => cumulative
                # 16/DMA targets are exact)
                nc.vector.wait_ge(dma_sems[ci % NQ], 16 * (ci // NQ + 1))

            # SUM over the packed [val|sq] rows: 4 bf16 halvings (16->1);
            # all stay in 2x perf mode (tensor_reduce would drop to 1x)
            a1 = tpool.tile([128, CH_T, 8, E2], BF, tag="a1")
            nc.vector.tensor_tensor(
                a1[:, :ntile], g4[:, :, 0:8, :], g4[:, :, 8:16, :], ALU.add)
            a2 = tpool.tile([128, CH_T, 4, E2], BF, tag="a2")
            nc.vector.tensor_tensor(
                a2[:, :ntile], a1[:, :ntile, 0:4, :], a1[:, :ntile, 4:8, :],
                ALU.add)
            a3 = tpool.tile([128, CH_T, 2, E2], BF, tag="a3")
            nc.vector.tensor_tensor(
                a3[:, :ntile], a2[:, :ntile, 0:2, :], a2[:, :ntile, 2:4, :],
                ALU.add)
            addf = tpool.tile([128, CH_T, E2], BF, tag="addf")
            nc.vector.tensor_tensor(
                addf[:, :ntile], a3[:, :ntile, 0, :], a3[:, :ntile, 1, :],
                ALU.add)

            featc = fpool.tile([128, CH_T, 4 * R], BF, tag="featc")

            # MAX/MIN: 4 bf16 TT halvings over the val halves
            def mmtree(op, dst_lo, tag):
                h1 = tpool.tile([128, CH_T, 8, R], BF, tag=tag + "1")
                nc.vector.tensor_tensor(
                    h1[:, :ntile], g4[:, :, 0:8, 0:R], g4[:, :, 8:16, 0:R], op)
                h2 = tpool.tile([128, CH_T, 4, R], BF, tag=tag + "2")
                nc.vector.tensor_tensor(
                    h2[:, :ntile], h1[:, :ntile, 0:4, :], h1[:, :ntile, 4:8, :],
                    op)
                h3 = tpool.tile([128, CH_T, 2, R], BF, tag=tag + "3")
                nc.vector.tensor_tensor(
                    h3[:, :ntile], h2[:, :ntile, 0:2, :], h2[:, :ntile, 2:4, :],
                    op)
                nc.vector.tensor_tensor(
                    featc[:, :ntile, dst_lo : dst_lo + R],
                    h3[:, :ntile, 0, :], h3[:, :ntile, 1, :], op)

            mmtree(ALU.max, R, "mx")       # max -> featc[:, :, 64:128]
            mmtree(ALU.min, 2 * R, "mn")   # min -> featc[:, :, 128:192]

            # mean / sq-mean on ACT (per-tile 1/len scale); std via sqrt+eps
            sqm = fpool.tile([128, CH_T, R], F32, tag="sqm")
            for tt in range(ntile):
                t = t0 + tt
                invl = aux_sb[:, t : t + 1]
                nc.scalar.activation(
                    featc[:, tt, 0:R], addf[:, tt, 0:R], AF.Copy, scale=invl)
                nc.scalar.activation(
                    sqm[:, tt, :], addf[:, tt, R:E2], AF.Copy, scale=invl)
            msq = fpool.tile([128, CH_T, R], F32, tag="msq")
            nc.scalar.activation(
                msq[:, :ntile], featc[:, :ntile, 0:R], AF.Square)
            nc.vector.tensor_tensor(
                sqm[:, :ntile], sqm[:, :ntile], msq[:, :ntile], ALU.subtract)
            nc.scalar.activation(
                featc[:, :ntile, 3 * R : 4 * R], sqm[:, :ntile], AF.Sqrt,
                bias=eps_sb[:, 0:1])

            # previous chunk's combine/normalize
            if state["pending"] is not None:
                finish(*state["pending"])

            # FC per tile: G_k = features @ W_k.T via PE-transposed features
            gps = psG.tile([128, CH_T, 3 * R], F32, tag="gp", name=f"gp_{t0}")
            for tt in range(ntile):
                fts = []
                for kc in range(2):
                    ftp = psF.tile([128, 128], BF, tag="ftp")
                    nc.tensor.transpose(
                        ftp[:], featc[:, tt, kc * 128 : (kc + 1) * 128],
                        identb_sb[:],
                    )
                    ft = fpool.tile([128, 128], BF, tag=f"fts{kc}")
                    nc.scalar.activation(ft[:], ftp[:], AF.Copy)
                    fts.append(ft)
                # complete each G_k's accumulation group before the next
                for k in range(3):
                    for kc in range(2):
                        nc.tensor.matmul(
                            gps[:, tt, k * R : (k + 1) * R],
                            fts[kc][:],
                            wret_sb[:, kc, k * R : (k + 1) * R],
                            start=(kc == 0),
                            stop=(kc == 1),
                        )

            state["pending"] = (t0, ntile, gps)

        # --- emission schedule ---
        n = len(CHUNKS)
        if not prep_mode:
            # non-prepared gathers read rtab2 at desc-gen time: the AG must
            # precede the first gather in the gpsimd stream
            emit_ag()
            half_done = False
            for ci in range(n):
                emit_prep(ci)
                emit_pool(ci)
                # once finish() has produced tiles [0, 24), stream them out
                if not half_done and ci > 0 and CHUNKS[ci - 1][0] >= 24:
                    nc.sync.dma_start(
                        out_ap[:, 0:24, :], ostage[:, 0:24, :])
                    half_done = True
        else:
            # Prepared desc-gen overlaps phase A (only idx_sb is read).
            # Ring capacity allows <=3 untriggered 4096-idx preps per queue
            # (HW-probed); gather-buf reuse caps outstanding preps at nbuf.
            # p0..p10, AG, T x4 (count=None: fires 0..10; its AG-done wait
            # is the one blocking point), then rounds of 4 preps followed
            # by 4 count=None triggers: each trigger's auto-wait (newest
            # covered prep's completion) hides under the other Q7 pairs'
            # desc-gen, so the pipeline keeps the 4-way rate while every
            # trigger is soundly ordered by a real semaphore wait.
            FIRST = min(nbuf, 3 * NQ - 1)   # 11
            active = stage in ("gather", "full")
            gate_state = {"n": 0}
            probe = fpool.tile([128, 8], BF, tag="agprobe")
            gsem = nc.alloc_semaphore(f"gate{_GATE_SEQ()}")

            def gate():
                # A gpsimd copy reading the AG probe completes only after
                # (a) the AllGather landed (RAW via the probe DMA) and
                # (b) every Q7 pair drained its queued desc-gen (all-core
                # execution, per-core in-order). then_inc + wait_ge turns
                # that into a sequencer-level barrier, so the triggers
                # behind it can never fire into half-written rings or a
                # half-written table.
                pc = fpool.tile([128, 8], BF, tag="pc")
                gate_state["n"] += 1
                nc.gpsimd.tensor_copy(pc[:], probe[:]).then_inc(gsem, 1)
                nc.gpsimd.wait_ge(gsem, gate_state["n"])

            for ci in range(FIRST):
                emit_prep(ci)
            emit_ag()
            if active:
                nc.sync.dma_start(probe[:], rtab2[0:128, 0:8])
                gate()
                for q in range(NQ):
                    nc.gpsimd.trigger_dma(count=None, queue_num=q)
            for cj in range(FIRST):
                emit_pool(cj)
            ci = FIRST
            while ci < n:
                grp = list(range(ci, min(ci + NQ, n)))
                for c in grp:
                    emit_prep(c)
                if active:
                    gate()
                    for c in grp:
                        nc.gpsimd.trigger_dma(count=None, queue_num=c % NQ)
                for c in grp:
                    emit_pool(c)
                ci += NQ

        if stage == "full" and state["pending"] is not None:
            finish(*state["pending"])


def build_kernel():
    nc = bacc.Bacc(
        "TRN2",
        target_bir_lowering=False,
        debug=False,
        num_devices=NCORES,
        num_swdge_queues=NQ,
    )
    embt = nc.declare_dram_parameter("embt", [HID, VSHP], BF, isOutput=False)
    wdt = nc.declare_dram_parameter("wdt", [HID, R], BF, isOutput=False)
    idx = nc.declare_dram_parameter("idx", [128, BPAD], I16, isOutput=False)
    aux = nc.declare_dram_parameter("aux", [128, 3 * NT], F32, isOutput=False)
    wret = nc.declare_dram_parameter("wret", [2, 128, 3 * R], BF, isOutput=False)
    biasr = nc.declare_dram_parameter("biasr", [128, R], F32, isOutput=False)
    ident = nc.declare_dram_parameter("ident", [128, 128], F32, isOutput=False)
    out = nc.declare_dram_parameter("out", [BPAD, R], F32, isOutput=True)

    with tile.TileContext(nc) as tc:
        with (
            tc.tile_pool(name="dram", bufs=1, space="DRAM") as dpool,
            tc.tile_pool(name="const", bufs=1) as cpool,
        ):
            rloc2 = dpool.tile([VSHP, E2], BF)
            rtab2 = dpool.tile([VOCABP, E2], BF, addr_space="Shared")

            wdt_sb = cpool.tile([128, KC, R], BF)
            nc.sync.dma_start(wdt_sb[:], wdt.rearrange("(k p) n -> p k n", p=128))
            idx_sb = cpool.tile([128, BPAD], I16)
            nc.sync.dma_start(idx_sb[:], idx[:])
            aux_sb = cpool.tile([128, 3 * NT], F32)
            nc.sync.dma_start(aux_sb[:], aux[:])
            wret_raw = cpool.tile([128, 2, 3 * R], BF)
            nc.sync.dma_start(wret_raw[:], wret.rearrange("c p n -> p c n"))
            wret_sb = cpool.tile([128, 2, 3 * R], BF)
            nc.scalar.activation(wret_sb[:], wret_raw[:], AF.Copy)
            biasr_sb = cpool.tile([128, R], F32)
            nc.sync.dma_start(biasr_sb[:], biasr[:])
            ident_sb = cpool.tile([128, 128], F32)
            nc.sync.dma_start(ident_sb[:], ident[:])
            ostage = cpool.tile([128, NT, R], F32)

            # identity staged through ACT so PE transposes dep on ACT sem only
            ident2_sb = cpool.tile([128, 128], F32)
            nc.scalar.activation(ident2_sb[:], ident_sb[:], AF.Copy)
            identb_sb = cpool.tile([128, 128], BF)
            nc.scalar.activation(identb_sb[:], ident_sb[:], AF.Copy)
            eps_sb = cpool.tile([128, 1], F32)
            nc.gpsimd.memset(eps_sb[:], 1e-6)

            # cross-core barrier: a tiny AllGather aligns the 8 cores early
            # so the real AllGather's peer-skew wait (~27us) disappears
            barr_in = dpool.tile([16, 8], F32)
            barr_out = dpool.tile([128, 8], F32)
            barr_sb = cpool.tile([16, 8], F32)
            nc.gpsimd.memset(barr_sb[:], 0.0)
            nc.sync.dma_start(barr_in[:], barr_sb[:])
            nc.gpsimd.collective_compute(
                "AllGather",
                ALU.bypass,
                replica_groups=[list(range(NCORES))],
                ins=[barr_in.opt()],
                outs=[barr_out.opt()],
            )

            # ---- Phase A: RtabT = W_downT.T @ embT (bf16) ----
            for _rep in range(int(os.environ.get("KREPS", "1"))):
              with (
                  tc.tile_pool(name="emb", bufs=3) as epool,
                  tc.tile_pool(name="stageA", bufs=1) as apool,
              ):
                  rtabT_sb = apool.tile([64, VSHP], F32)
                  rloc2_sb = apool.tile([128, VSHP // 128, E2], BF)
                  with tc.tile_pool(name="psA", bufs=1, space="PSUM") as psA:
                      rtabT_ps = psA.tile([64, VSHP], F32)
                      # gate: junk matmul reading only wdt_sb -> absorbs
                      # the wdt DMA-lane wait so real matmuls carry just
                      # their ech lane
                      nc.tensor.matmul(
                          rtabT_ps[:, VSHP - 64 : VSHP - 32],
                          wdt_sb[:, 0, :],
                          wdt_sb[:, 0, 0:32],
                          start=True,
                          stop=True,
                          skip_group_check=True,
                      )
                      for k in range(KC):
                          ech = epool.tile([128, VSHP], BF, tag="ech")
                          nc.sync.dma_start(
                              ech[:], embt[k * 128 : (k + 1) * 128, :])
                          for vb in range(VSHP // 512):
                              c0 = vb * 512
                              c1 = min((vb + 1) * 512, VSHP - 64)
                              nc.tensor.matmul(
                                  rtabT_ps[:, c0:c1],
                                  wdt_sb[:, k, :],
                                  ech[:, c0:c1],
                                  start=(k == 0),
                                  stop=(k == KC - 1),
                              )
                      # absorber: junk matmul into the other pad half
                      # carries the PSUM drain wait (Matmult = 1 wait max)
                      nc.tensor.matmul(
                          rtabT_ps[:, VSHP - 32 : VSHP],
                          wdt_sb[:, 0, :],
                          wdt_sb[:, 0, 32:64],
                          start=True,
                          stop=True,
                          skip_group_check=True,
                      )
                      nc.scalar.activation(rtabT_sb[:], rtabT_ps[:], AF.Copy)

                  with tc.tile_pool(name="psT", bufs=2, space="PSUM") as psT:
                      # dummy junk matmul: carries the psA->psT drain wait
                      dtp = psT.tile([64, 64], F32, tag="tp")
                      nc.tensor.matmul(
                          dtp[:], wdt_sb[:, 0, :], wdt_sb[:, 0, :],
                          start=True, stop=True,
                      )
                      nc.scalar.activation(
                          ostage[0:64, NT - 1, :], dtp[:], AF.Copy)
                      for v in range(VSHP // 128):
                          tp = psT.tile([128, 64], F32, tag="tp")
                          nc.tensor.transpose(
                              tp[:],
                              rtabT_sb[:, v * 128 : (v + 1) * 128],
                              ident2_sb[:64, :64],
                          )
                          nc.scalar.activation(
                              rloc2_sb[:, v, 0:R], tp[:], AF.Copy)
                          nc.scalar.activation(
                              rloc2_sb[:, v, R:E2], tp[:], AF.Square)
                      nc.sync.dma_start(
                          rloc2.rearrange("(v p) n -> p v n", p=128), rloc2_sb[:]
                      )

                      def emit_ag():
                          # ---- Phase B: AllGather rloc2 -> rtab2 ----
                          nc.gpsimd.collective_compute(
                              "AllGather",
                              ALU.bypass,
                              replica_groups=[list(range(NCORES))],
                              ins=[rloc2.opt()],
                              outs=[rtab2.opt()],
                          )

                      # ---- Phase C: gather + pool + FC ----
                      out_ap = out.rearrange("(t p) n -> p t n", p=128)
                      _phase_c(nc, tc, psT, rtab2, idx_sb, aux_sb, wret_sb,
                               biasr_sb, identb_sb, ostage, eps_sb, emit_ag,
                               out_ap)

                      nc.sync.dma_start(
                          out_ap[:, 24:NT, :], ostage[:, 24:NT, :]
                      )

    nc.compile()
    return nc


_NC_CACHE = {}


def _get_nc():
    key = (
        os.environ.get("KREPS", "1"),
        os.environ.get("KSTAGE", "full"),
        os.environ.get("KPREP", "0"),
        os.environ.get("KBUFS", "11"),
    )
    if key not in _NC_CACHE:
        _NC_CACHE[key] = build_kernel()
    return _NC_CACHE[key]


def _prepare(text_embeddings, kgl2token, W_down, W_re, b_re):
    import ml_dtypes

    emb = np.asarray(text_embeddings, dtype=np.float32)
    ids = np.asarray(kgl2token)
    wd = np.asarray(W_down, dtype=np.float32)
    wr = np.asarray(W_re, dtype=np.float32)
    br = np.asarray(b_re, dtype=np.float32)

    # host-side scalars: lengths and scale factors (global mean over all rows)
    lengths = (ids > 0).sum(axis=1).astype(np.float32)  # [B]
    scale = np.log(lengths + 0.0)
    scale = scale / (scale.mean() + 1e-10)
    iscale = 1.0 / np.clip(scale, 0.01, None)
    invl = (1.0 / (lengths + 1e-10)).astype(np.float32)

    # remap ids into padded vocab layout
    ids64 = ids.astype(np.int64)
    rid = (ids64 // VSH) * VSHP + (ids64 % VSH)  # [B, S] < 32768

    wdt = np.ascontiguousarray(wd.T).astype(ml_dtypes.bfloat16)  # [4096, 64]

    # W_re: result index = feat*3 + k  ->  W_k = W_re[:, k::3]  [64, 256]
    wret = np.zeros((2, 128, 3 * R), dtype=np.float32)
    for k in range(3):
        wkT = np.ascontiguousarray(wr[:, k::3].T)  # [256, 64]
        for kc in range(2):
            wret[kc, :, k * R : (k + 1) * R] = wkT[kc * 128 : (kc + 1) * 128, :]
    wret = wret.astype(ml_dtypes.bfloat16)
    biasr = np.tile(br[None, :], (128, 1)).astype(np.float32)
    identm = np.eye(128, dtype=np.float32)

    in_maps = []
    for c in range(NCORES):
        embt = np.zeros((HID, VSHP), dtype=ml_dtypes.bfloat16)
        embt[:, :VSH] = emb[c * VSH : (c + 1) * VSH, :].T.astype(ml_dtypes.bfloat16)
        # per-core padded rows
        rid_c = np.zeros((BPAD, S), dtype=np.int64)
        rid_c[:BSH] = rid[c * BSH : (c + 1) * BSH]
        # gather order: j = t*2048 + s*128 + r
        L = rid_c.reshape(NT, 128, S).transpose(0, 2, 1).reshape(-1)  # [BPAD*S]
        idx16 = L.reshape(-1, 16).T.astype(np.int16)  # [16, BPAD]
        idxsb = np.ascontiguousarray(np.tile(idx16, (8, 1)))  # [128, BPAD]

        auxc = np.zeros((128, 3 * NT), dtype=np.float32)
        for name_i, v in enumerate((invl, scale, iscale)):
            vc = np.ones(BPAD, dtype=np.float32)
            vc[:BSH] = v[c * BSH : (c + 1) * BSH]
            auxc[:, name_i * NT : (name_i + 1) * NT] = vc.reshape(NT, 128).T
        in_maps.append(
            dict(embt=embt, wdt=wdt, idx=idxsb, aux=auxc, wret=wret,
                 biasr=biasr, ident=identm)
        )
    return in_maps, lengths, scale, iscale, invl


def _patch_rows(result, text_embeddings, kgl2token, W_down, W_re, b_re,
                scale_all, iscale_all, invl_all):
    """Recompute rows containing any id==0 token exactly (host, numpy)."""
    ids = np.asarray(kgl2token)
    bad = np.nonzero((ids <= 0).any(axis=1))[0]
    if len(bad) == 0:
        return result
    emb = np.asarray(text_embeddings, dtype=np.float32)
    wd = np.asarray(W_down, dtype=np.float32)
    wr = np.asarray(W_re, dtype=np.float32)
    br = np.asarray(b_re, dtype=np.float32)
    for r in bad:
        tok_ids = ids[r].astype(np.int64)
        tok = emb[tok_ids] @ wd.T  # [S, R]
        mask = (tok_ids > 0).astype(np.float32)[:, None]
        length = mask.sum()
        masked = tok * mask
        mean = masked.sum(axis=0) / (length + 1e-10)
        sq_mean = (tok * tok * mask).sum(axis=0) / (length + 1e-10)
        mx = (masked + (1.0 - mask) * (-1e10)).max(axis=0)
        mn = (masked + (1.0 - mask) * (1e10)).min(axis=0)
        std = np.sqrt(np.clip(sq_mean - mean * mean, 1e-6, None))
        features = np.concatenate([mean, mx, mn, std])  # [256]
        scales = np.array([1.0, scale_all[r], iscale_all[r]], dtype=np.float32)
        flat = (features[:, None] * scales[None, :]).reshape(-1)  # [768]
        res = flat @ wr.T + br
        nrm = np.linalg.norm(res)
        result[r] = res / max(nrm, 1e-12)
    return result


def kernel(text_embeddings, kgl2token, W_down, W_re, b_re, _trace=False):
    nc = _get_nc()
    in_maps, lengths, scale, iscale, invl = _prepare(
        text_embeddings, kgl2token, W_down, W_re, b_re
    )
    r = run_bass_kernel_spmd(nc, in_maps, core_ids=list(range(NCORES)), trace=_trace)
    outs = [r.results[c]["out"][:BSH] for c in range(NCORES)]
    result = np.concatenate(outs, axis=0).astype(np.float32)
    result = _patch_rows(
        result, text_embeddings, kgl2token, W_down, W_re, b_re, scale, iscale, invl
    )
    if _trace:
        return result, r
    return result


# revision 30
# speedup vs baseline: 1.0232x; 1.0232x over previous
"""BasePNARetriever Trainium2 kernel (8 NeuronCores, SPMD).

Strategy (v6, ~2.1x over the 946us v2 baseline):
  - A tiny AllGather at t~0 acts as a cross-core barrier so the real
    AllGather's peer-skew wait (~27us measured) mostly disappears.
  - Phase A (vocab-sharded down-projection) in bf16: each core streams a
    [4096, 4096] bf16 slice of text_embeddings via HWDGE (nc.sync - NOT
    SWDGE, so emb loads never queue behind gather desc-gen on GpSimd) and
    computes RtabT[64, 4096] on PE (bf16 matmul, fp32 PSUM accumulate).
    PE-transposes back to row-major; ACT emits the bf16 table slice with
    rows packed [val(64) | val^2(64)] (256B rows: the squares ride the
    gather for free). One AllGather builds rtab2[32768, 128] bf16 in DRAM.
  - Phase C: gather descriptor generation is the wall. Measured on HW:
    7.9 ns/descriptor per Q7 pair, but instructions on DIFFERENT SWDGE
    queues overlap across the four Q7 pairs (~2 ns/desc effective), so
    the 25 chunk-gathers round-robin queues 0-3. The gathers follow the
    AllGather (RAW on rtab2) and desc-gen runs concurrently with pooling.
  - Pooling per 2-tile chunk, engine-partitioned to avoid the DVE-2-port
    vs Q7-descriptor-ring SBUF lock: DVE runs ONLY tensor_tensor trees
    (never contends): sum via 4 bf16 halvings, max/min via 4 bf16
    halvings each; ACT (never contends) does mean/sq-mean scaling with
    per-row activation(scale=...), squares, std sqrt(+eps bias), the
    G1/G2 scale combines, and Square+accum_out for the L2 row norms; PE
    does the FC via PE-transposed bf16 features.
  - Host precomputes lengths/log-scales and patches the rare rows
    containing id==0 tokens (~25 rows in 50000).
  - KPREP=1 (experimental, off by default) additionally pre-generates
    gather descriptors during phase A via prepare_only + trigger_dma;
    it overlaps desc-gen fully (-70us) but currently returns corrupted
    data - the prep-DMA completion semaphores are not correctly threaded
    to the pooling readers.
"""

import sys

sys.path.insert(0, "/opt/trn_rl_repo")

import os

import numpy as np

import concourse.bass as bass
import concourse.bacc as bacc
import concourse.mybir as mybir
import concourse.tile as tile
from concourse.bass_utils import run_bass_kernel_spmd

F32 = mybir.dt.float32
BF = mybir.dt.bfloat16
I16 = mybir.dt.int16
AF = mybir.ActivationFunctionType
ALU = mybir.AluOpType

NCORES = 8
VOCAB, HID, R, B, S = 32000, 4096, 64, 50000, 16
VSH = VOCAB // NCORES          # 4000 real vocab rows per core
VSHP = 4096                    # padded vocab rows per core (32 x 128)
VOCABP = VSHP * NCORES         # 32768 padded vocab
KC = HID // 128                # 32 contraction chunks
BSH = B // NCORES              # 6250 rows per core
NT = 49                        # row tiles of 128 (6272 padded rows)
BPAD = NT * 128                # 6272
E2 = 2 * R                     # 128: packed table row [val(64)|sq(64)]
CH_T = 2                       # row-tiles per gather chunk
CHUNKS = [(i, min(CH_T, NT - i)) for i in range(0, NT, CH_T)]  # (tile0, ntiles)
NQ = 4                         # SWDGE queues
_GATE_CTR = [0]


def _GATE_SEQ():
    _GATE_CTR[0] += 1
    return _GATE_CTR[0]

AG_AT = 8                      # emit the AllGather after this many preps
TRIG_AT = 12                   # first triggers after this many preps


def _phase_c(nc, tc, psT, rtab2, idx_sb, aux_sb, wret_sb, biasr_sb, identb_sb,
             ostage, eps_sb, emit_ag):
    stage = os.environ.get("KSTAGE", "full")
    prep_mode = os.environ.get("KPREP", "0") == "1"
    nbuf = int(os.environ.get("KBUFS", "11"))
    with (
        tc.tile_pool(name="g", bufs=nbuf) as gpool,
        tc.tile_pool(name="tr", bufs=2) as tpool,
        tc.tile_pool(name="f", bufs=2) as fpool,
        tc.tile_pool(name="psG", bufs=2, space="PSUM") as psG,
    ):
        psF = psT  # reuse the open pool: bank history already PE-observed
        dma_sems = (
            [nc.alloc_semaphore(f"gsem{q}") for q in range(NQ)] if prep_mode else None
        )
        gtiles = {}
        state = {"pending": None}

        def emit_prep(ci):
            t0, ntile = CHUNKS[ci]
            nidx = ntile * 2048
            nslot = ntile * 16
            g = gpool.tile([128, CH_T * 16, E2], BF, tag="g")
            q = ci % NQ
            if stage in ("gather", "full"):
                kw = {}
                if prep_mode:
                    kw = dict(prepare_only=True, sem=dma_sems[q])
                nc.gpsimd.dma_gather(
                    g[:, :nslot, :],
                    rtab2[:],
                    idx_sb[:, t0 * 128 : t0 * 128 + nidx // 16],
                    nidx,
                    nidx,
                    E2,
                    single_packet=False,
                    queue_num=q,
                    **kw,
                )
            gtiles[ci] = g

        def emit_trigger(ci):
            if prep_mode and stage in ("gather", "full"):
                nc.gpsimd.trigger_dma(count=None, queue_num=ci % NQ)

        def finish(t0, ntile, gps):
            # res = G0 + G1*scale + G2*iscale + bias, then L2 normalize.
            # Deferred one chunk so the DVE queue never stalls on the PE/ACT
            # round-trip that produces gps.
            res = fpool.tile([128, CH_T, R], F32, tag="res")
            r2 = fpool.tile([128, CH_T, R], F32, tag="r2")
            for tt in range(ntile):
                t = t0 + tt
                nc.scalar.activation(
                    res[:, tt, :], gps[:, tt, R : 2 * R], AF.Copy,
                    scale=aux_sb[:, NT + t : NT + t + 1])
                nc.scalar.activation(
                    r2[:, tt, :], gps[:, tt, 2 * R : 3 * R], AF.Copy,
                    scale=aux_sb[:, 2 * NT + t : 2 * NT + t + 1])
            nc.vector.tensor_tensor(
                res[:, :ntile], res[:, :ntile], gps[:, :ntile, 0:R], ALU.add)
            nc.vector.tensor_add(res[:, :ntile], res[:, :ntile], r2[:, :ntile])
            nc.vector.tensor_tensor(
                ostage[:, t0 : t0 + ntile, :], res[:, :ntile],
                biasr_sb[:, None, :].broadcast_to([128, ntile, R]), ALU.add)
            # L2 norm: ACT Square + accum_out gives the row sum of squares
            sqr = fpool.tile([128, CH_T, R], F32, tag="sqr")
            ss = fpool.tile([128, CH_T], F32, tag="ss")
            for tt in range(ntile):
                nc.scalar.activation(
                    sqr[:, tt, :], ostage[:, t0 + tt, :], AF.Square,
                    accum_out=ss[:, tt : tt + 1])
            iss = fpool.tile([128, CH_T], F32, tag="iss")
            nc.vector.reciprocal(iss[:, :ntile], ss[:, :ntile])
            rin = fpool.tile([128, CH_T], F32, tag="rin")
            nc.scalar.sqrt(rin[:, :ntile], iss[:, :ntile])
            nc.vector.tensor_mul(
                ostage[:, t0 : t0 + ntile, :], ostage[:, t0 : t0 + ntile, :],
                rin[:, :ntile][:, :, None].broadcast_to([128, ntile, R]))

        def emit_pool(ci):
            if stage != "full":
                return
            t0, ntile = CHUNKS[ci]
            g = gtiles.pop(ci)
            nslot = ntile * 16
            g4 = g[:, :nslot, :].rearrange("p (t s) e -> p t s e", s=16)

            if prep_mode:
                # all direct readers of g are DVE ops; gate the DVE stream on
                # the chunk's DMA completion (per-queue FIFO => cumulative
                # 16/DMA targets are exact)
                nc.vector.wait_ge(dma_sems[ci % NQ], 16 * (ci // NQ + 1))

            # SUM over the packed [val|sq] rows: 4 bf16 halvings (16->1);
            # all stay in 2x perf mode (tensor_reduce would drop to 1x)
            a1 = tpool.tile([128, CH_T, 8, E2], BF, tag="a1")
            nc.vector.tensor_tensor(
                a1[:, :ntile], g4[:, :, 0:8, :], g4[:, :, 8:16, :], ALU.add)
            a2 = tpool.tile([128, CH_T, 4, E2], BF, tag="a2")
            nc.vector.tensor_tensor(
                a2[:, :ntile], a1[:, :ntile, 0:4, :], a1[:, :ntile, 4:8, :],
                ALU.add)
            a3 = tpool.tile([128, CH_T, 2, E2], BF, tag="a3")
            nc.vector.tensor_tensor(
                a3[:, :ntile], a2[:, :ntile, 0:2, :], a2[:, :ntile, 2:4, :],
                ALU.add)
            addf = tpool.tile([128, CH_T, E2], BF, tag="addf")
            nc.vector.tensor_tensor(
                addf[:, :ntile], a3[:, :ntile, 0, :], a3[:, :ntile, 1, :],
                ALU.add)

            featc = fpool.tile([128, CH_T, 4 * R], BF, tag="featc")

            # MAX/MIN: 4 bf16 TT halvings over the val halves
            def mmtree(op, dst_lo, tag):
                h1 = tpool.tile([128, CH_T, 8, R], BF, tag=tag + "1")
                nc.vector.tensor_tensor(
                    h1[:, :ntile], g4[:, :, 0:8, 0:R], g4[:, :, 8:16, 0:R], op)
                h2 = tpool.tile([128, CH_T, 4, R], BF, tag=tag + "2")
                nc.vector.tensor_tensor(
                    h2[:, :ntile], h1[:, :ntile, 0:4, :], h1[:, :ntile, 4:8, :],
                    op)
                h3 = tpool.tile([128, CH_T, 2, R], BF, tag=tag + "3")
                nc.vector.tensor_tensor(
                    h3[:, :ntile], h2[:, :ntile, 0:2, :], h2[:, :ntile, 2:4, :],
                    op)
                nc.vector.tensor_tensor(
                    featc[:, :ntile, dst_lo : dst_lo + R],
                    h3[:, :ntile, 0, :], h3[:, :ntile, 1, :], op)

            mmtree(ALU.max, R, "mx")       # max -> featc[:, :, 64:128]
            mmtree(ALU.min, 2 * R, "mn")   # min -> featc[:, :, 128:192]

            # mean / sq-mean on ACT (per-tile 1/len scale); std via sqrt+eps
            sqm = fpool.tile([128, CH_T, R], F32, tag="sqm")
            for tt in range(ntile):
                t = t0 + tt
                invl = aux_sb[:, t : t + 1]
                nc.scalar.activation(
                    featc[:, tt, 0:R], addf[:, tt, 0:R], AF.Copy, scale=invl)
                nc.scalar.activation(
                    sqm[:, tt, :], addf[:, tt, R:E2], AF.Copy, scale=invl)
            msq = fpool.tile([128, CH_T, R], F32, tag="msq")
            nc.scalar.activation(
                msq[:, :ntile], featc[:, :ntile, 0:R], AF.Square)
            nc.vector.tensor_tensor(
                sqm[:, :ntile], sqm[:, :ntile], msq[:, :ntile], ALU.subtract)
            nc.scalar.activation(
                featc[:, :ntile, 3 * R : 4 * R], sqm[:, :ntile], AF.Sqrt,
                bias=eps_sb[:, 0:1])

            # previous chunk's combine/normalize
            if state["pending"] is not None:
                finish(*state["pending"])

            # FC per tile: G_k = features @ W_k.T via PE-transposed features
            gps = psG.tile([128, CH_T, 3 * R], F32, tag="gp", name=f"gp_{t0}")
            for tt in range(ntile):
                fts = []
                for kc in range(2):
                    ftp = psF.tile([128, 128], BF, tag="ftp")
                    nc.tensor.transpose(
                        ftp[:], featc[:, tt, kc * 128 : (kc + 1) * 128],
                        identb_sb[:],
                    )
                    ft = fpool.tile([128, 128], BF, tag=f"fts{kc}")
                    nc.scalar.activation(ft[:], ftp[:], AF.Copy)
                    fts.append(ft)
                # complete each G_k's accumulation group before the next
                for k in range(3):
                    for kc in range(2):
                        nc.tensor.matmul(
                            gps[:, tt, k * R : (k + 1) * R],
                            fts[kc][:],
                            wret_sb[:, kc, k * R : (k + 1) * R],
                            start=(kc == 0),
                            stop=(kc == 1),
                        )

            state["pending"] = (t0, ntile, gps)

        # --- emission schedule ---
        n = len(CHUNKS)
        if not prep_mode:
            # non-prepared gathers read rtab2 at desc-gen time: the AG must
            # precede the first gather in the gpsimd stream
            emit_ag()
            for ci in range(n):
                emit_prep(ci)
                emit_pool(ci)
        else:
            # Prepared desc-gen overlaps phase A (only idx_sb is read).
            # Ring capacity allows <=3 untriggered 4096-idx preps per queue
            # (HW-probed); gather-buf reuse caps outstanding preps at nbuf.
            # p0..p10, AG, T x4 (count=None: fires 0..10; its AG-done wait
            # is the one blocking point), then rounds of 4 preps followed
            # by 4 count=None triggers: each trigger's auto-wait (newest
            # covered prep's completion) hides under the other Q7 pairs'
            # desc-gen, so the pipeline keeps the 4-way rate while every
            # trigger is soundly ordered by a real semaphore wait.
            FIRST = min(nbuf, 3 * NQ - 1)   # 11
            active = stage in ("gather", "full")
            gate_state = {"n": 0}
            probe = fpool.tile([128, 8], BF, tag="agprobe")
            gsem = nc.alloc_semaphore(f"gate{_GATE_SEQ()}")

            def gate():
                # A gpsimd copy reading the AG probe completes only after
                # (a) the AllGather landed (RAW via the probe DMA) and
                # (b) every Q7 pair drained its queued desc-gen (all-core
                # execution, per-core in-order). then_inc + wait_ge turns
                # that into a sequencer-level barrier, so the triggers
                # behind it can never fire into half-written rings or a
                # half-written table.
                pc = fpool.tile([128, 8], BF, tag="pc")
                gate_state["n"] += 1
                nc.gpsimd.tensor_copy(pc[:], probe[:]).then_inc(gsem, 1)
                nc.gpsimd.wait_ge(gsem, gate_state["n"])

            for ci in range(FIRST):
                emit_prep(ci)
            emit_ag()
            if active:
                nc.sync.dma_start(probe[:], rtab2[0:128, 0:8])
                gate()
                for q in range(NQ):
                    nc.gpsimd.trigger_dma(count=None, queue_num=q)
            for cj in range(FIRST):
                emit_pool(cj)
            ci = FIRST
            while ci < n:
                grp = list(range(ci, min(ci + NQ, n)))
                for c in grp:
                    emit_prep(c)
                if active:
                    gate()
                    for c in grp:
                        nc.gpsimd.trigger_dma(count=None, queue_num=c % NQ)
                for c in grp:
                    emit_pool(c)
                ci += NQ

        if stage == "full" and state["pending"] is not None:
            finish(*state["pending"])


def build_kernel():
    nc = bacc.Bacc(
        "TRN2",
        target_bir_lowering=False,
        debug=False,
        num_devices=NCORES,
        num_swdge_queues=NQ,
    )
    embt = nc.declare_dram_parameter("embt", [HID, VSHP], BF, isOutput=False)
    wdt = nc.declare_dram_parameter("wdt", [HID, R], BF, isOutput=False)
    idx = nc.declare_dram_parameter("idx", [128, BPAD], I16, isOutput=False)
    aux = nc.declare_dram_parameter("aux", [128, 3 * NT], F32, isOutput=False)
    wret = nc.declare_dram_parameter("wret", [2, 128, 3 * R], BF, isOutput=False)
    biasr = nc.declare_dram_parameter("biasr", [128, R], F32, isOutput=False)
    ident = nc.declare_dram_parameter("ident", [128, 128], F32, isOutput=False)
    out = nc.declare_dram_parameter("out", [BPAD, R], F32, isOutput=True)

    with tile.TileContext(nc) as tc:
        with (
            tc.tile_pool(name="dram", bufs=1, space="DRAM") as dpool,
            tc.tile_pool(name="const", bufs=1) as cpool,
        ):
            rloc2 = dpool.tile([VSHP, E2], BF)
            rtab2 = dpool.tile([VOCABP, E2], BF, addr_space="Shared")

            wdt_sb = cpool.tile([128, KC, R], BF)
            nc.sync.dma_start(wdt_sb[:], wdt.rearrange("(k p) n -> p k n", p=128))
            idx_sb = cpool.tile([128, BPAD], I16)
            nc.sync.dma_start(idx_sb[:], idx[:])
            aux_sb = cpool.tile([128, 3 * NT], F32)
            nc.sync.dma_start(aux_sb[:], aux[:])
            wret_raw = cpool.tile([128, 2, 3 * R], BF)
            nc.sync.dma_start(wret_raw[:], wret.rearrange("c p n -> p c n"))
            wret_sb = cpool.tile([128, 2, 3 * R], BF)
            nc.scalar.activation(wret_sb[:], wret_raw[:], AF.Copy)
            biasr_sb = cpool.tile([128, R], F32)
            nc.sync.dma_start(biasr_sb[:], biasr[:])
            ident_sb = cpool.tile([128, 128], F32)
            nc.sync.dma_start(ident_sb[:], ident[:])
            ostage = cpool.tile([128, NT, R], F32)

            # identity staged through ACT so PE transposes dep on ACT sem only
            ident2_sb = cpool.tile([128, 128], F32)
            nc.scalar.activation(ident2_sb[:], ident_sb[:], AF.Copy)
            identb_sb = cpool.tile([128, 128], BF)
            nc.scalar.activation(identb_sb[:], ident_sb[:], AF.Copy)
            eps_sb = cpool.tile([128, 1], F32)
            nc.gpsimd.memset(eps_sb[:], 1e-6)

            # cross-core barrier: a tiny AllGather aligns the 8 cores early
            # so the real AllGather's peer-skew wait (~27us) disappears
            barr_in = dpool.tile([16, 8], F32)
            barr_out = dpool.tile([128, 8], F32)
            barr_sb = cpool.tile([16, 8], F32)
            nc.gpsimd.memset(barr_sb[:], 0.0)
            nc.sync.dma_start(barr_in[:], barr_sb[:])
            nc.gpsimd.collective_compute(
                "AllGather",
                ALU.bypass,
                replica_groups=[list(range(NCORES))],
                ins=[barr_in.opt()],
                outs=[barr_out.opt()],
            )

            # ---- Phase A: RtabT = W_downT.T @ embT (bf16) ----
            for _rep in range(int(os.environ.get("KREPS", "1"))):
              with (
                  tc.tile_pool(name="emb", bufs=3) as epool,
                  tc.tile_pool(name="stageA", bufs=1) as apool,
              ):
                  rtabT_sb = apool.tile([64, VSHP], F32)
                  rloc2_sb = apool.tile([128, VSHP // 128, E2], BF)
                  with tc.tile_pool(name="psA", bufs=1, space="PSUM") as psA:
                      rtabT_ps = psA.tile([64, VSHP], F32)
                      # gate: junk matmul reading only wdt_sb -> absorbs
                      # the wdt DMA-lane wait so real matmuls carry just
                      # their ech lane
                      nc.tensor.matmul(
                          rtabT_ps[:, VSHP - 64 : VSHP - 32],
                          wdt_sb[:, 0, :],
                          wdt_sb[:, 0, 0:32],
                          start=True,
                          stop=True,
                          skip_group_check=True,
                      )
                      for k in range(KC):
                          ech = epool.tile([128, VSHP], BF, tag="ech")
                          nc.sync.dma_start(
                              ech[:], embt[k * 128 : (k + 1) * 128, :])
                          for vb in range(VSHP // 512):
                              c0 = vb * 512
                              c1 = min((vb + 1) * 512, VSHP - 64)
                              nc.tensor.matmul(
                                  rtabT_ps[:, c0:c1],
                                  wdt_sb[:, k, :],
                                  ech[:, c0:c1],
                                  start=(k == 0),
                                  stop=(k == KC - 1),
                              )
                      # absorber: junk matmul into the other pad half
                      # carries the PSUM drain wait (Matmult = 1 wait max)
                      nc.tensor.matmul(
                          rtabT_ps[:, VSHP - 32 : VSHP],
                          wdt_sb[:, 0, :],
                          wdt_sb[:, 0, 32:64],
                          start=True,
                          stop=True,
                          skip_group_check=True,
                      )
                      nc.scalar.activation(rtabT_sb[:], rtabT_ps[:], AF.Copy)

                  with tc.tile_pool(name="psT", bufs=2, space="PSUM") as psT:
                      # dummy junk matmul: carries the psA->psT drain wait
                      dtp = psT.tile([64, 64], F32, tag="tp")
                      nc.tensor.matmul(
                          dtp[:], wdt_sb[:, 0, :], wdt_sb[:, 0, :],
                          start=True, stop=True,
                      )
                      nc.scalar.activation(
                          ostage[0:64, NT - 1, :], dtp[:], AF.Copy)
                      for v in range(VSHP // 128):
                          tp = psT.tile([128, 64], F32, tag="tp")
                          nc.tensor.transpose(
                              tp[:],
                              rtabT_sb[:, v * 128 : (v + 1) * 128],
                              ident2_sb[:64, :64],
                          )
                          nc.scalar.activation(
                              rloc2_sb[:, v, 0:R], tp[:], AF.Copy)
                          nc.scalar.activation(
                              rloc2_sb[:, v, R:E2], tp[:], AF.Square)
                      nc.sync.dma_start(
                          rloc2.rearrange("(v p) n -> p v n", p=128), rloc2_sb[:]
                      )

                      def emit_ag():
                          # ---- Phase B: AllGather rloc2 -> rtab2 ----
                          nc.gpsimd.collective_compute(
                              "AllGather",
                              ALU.bypass,
                              replica_groups=[list(range(NCORES))],
                              ins=[rloc2.opt()],
                              outs=[rtab2.opt()],
                          )

                      # ---- Phase C: gather + pool + FC ----
                      _phase_c(nc, tc, psT, rtab2, idx_sb, aux_sb, wret_sb,
                               biasr_sb, identb_sb, ostage, eps_sb, emit_ag)

                      nc.sync.dma_start(
                          out.rearrange("(t p) n -> p t n", p=128), ostage[:]
                      )

    nc.compile()
    return nc


_NC_CACHE = {}


def _get_nc():
    key = (
        os.environ.get("KREPS", "1"),
        os.environ.get("KSTAGE", "full"),
        os.environ.get("KPREP", "0"),
        os.environ.get("KBUFS", "11"),
    )
    if key not in _NC_CACHE:
        _NC_CACHE[key] = build_kernel()
    return _NC_CACHE[key]


def _prepare(text_embeddings, kgl2token, W_down, W_re, b_re):
    import ml_dtypes

    emb = np.asarray(text_embeddings, dtype=np.float32)
    ids = np.asarray(kgl2token)
    wd = np.asarray(W_down, dtype=np.float32)
    wr = np.asarray(W_re, dtype=np.float32)
    br = np.asarray(b_re, dtype=np.float32)

    # host-side scalars: lengths and scale factors (global mean over all rows)
    lengths = (ids > 0).sum(axis=1).astype(np.float32)  # [B]
    scale = np.log(lengths + 0.0)
    scale = scale / (scale.mean() + 1e-10)
    iscale = 1.0 / np.clip(scale, 0.01, None)
    invl = (1.0 / (lengths + 1e-10)).astype(np.float32)

    # remap ids into padded vocab layout
    ids64 = ids.astype(np.int64)
    rid = (ids64 // VSH) * VSHP + (ids64 % VSH)  # [B, S] < 32768

    wdt = np.ascontiguousarray(wd.T).astype(ml_dtypes.bfloat16)  # [4096, 64]

    # W_re: result index = feat*3 + k  ->  W_k = W_re[:, k::3]  [64, 256]
    wret = np.zeros((2, 128, 3 * R), dtype=np.float32)
    for k in range(3):
        wkT = np.ascontiguousarray(wr[:, k::3].T)  # [256, 64]
        for kc in range(2):
            wret[kc, :, k * R : (k + 1) * R] = wkT[kc * 128 : (kc + 1) * 128, :]
    wret = wret.astype(ml_dtypes.bfloat16)
    biasr = np.tile(br[None, :], (128, 1)).astype(np.float32)
    identm = np.eye(128, dtype=np.float32)

    in_maps = []
    for c in range(NCORES):
        embt = np.zeros((HID, VSHP), dtype=ml_dtypes.bfloat16)
        embt[:, :VSH] = emb[c * VSH : (c + 1) * VSH, :].T.astype(ml_dtypes.bfloat16)
        # per-core padded rows
        rid_c = np.zeros((BPAD, S), dtype=np.int64)
        rid_c[:BSH] = rid[c * BSH : (c + 1) * BSH]
        # gather order: j = t*2048 + s*128 + r
        L = rid_c.reshape(NT, 128, S).transpose(0, 2, 1).reshape(-1)  # [BPAD*S]
        idx16 = L.reshape(-1, 16).T.astype(np.int16)  # [16, BPAD]
        idxsb = np.ascontiguousarray(np.tile(idx16, (8, 1)))  # [128, BPAD]

        auxc = np.zeros((128, 3 * NT), dtype=np.float32)
        for name_i, v in enumerate((invl, scale, iscale)):
            vc = np.ones(BPAD, dtype=np.float32)
            vc[:BSH] = v[c * BSH : (c + 1) * BSH]
            auxc[:, name_i * NT : (name_i + 1) * NT] = vc.reshape(NT, 128).T
        in_maps.append(
            dict(embt=embt, wdt=wdt, idx=idxsb, aux=auxc, wret=wret,
                 biasr=biasr, ident=identm)
        )
    return in_maps, lengths, scale, iscale, invl


def _patch_rows(result, text_embeddings, kgl2token, W_down, W_re, b_re,
                scale_all, iscale_all, invl_all):
    """Recompute rows containing any id==0 token exactly (host, numpy)."""
    ids = np.asarray(kgl2token)
    bad = np.nonzero((ids <= 0).any(axis=1))[0]
    if len(bad) == 0:
        return result
    emb = np.asarray(text_embeddings, dtype=np.float32)
    wd = np.asarray(W_down, dtype=np.float32)
    wr = np.asarray(W_re, dtype=np.float32)
    br = np.asarray(b_re, dtype=np.float32)
    for r in bad:
        tok_ids = ids[r].astype(np.int64)
        tok = emb[tok_ids] @ wd.T  # [S, R]
        mask = (tok_ids > 0).astype(np.float32)[:, None]
        length = mask.sum()
        masked = tok * mask
        mean = masked.sum(axis=0) / (length + 1e-10)
        sq_mean = (tok * tok * mask).sum(axis=0) / (length + 1e-10)
        mx = (masked + (1.0 - mask) * (-1e10)).max(axis=0)
        mn = (masked + (1.0 - mask) * (1e10)).min(axis=0)
        std = np.sqrt(np.clip(sq_mean - mean * mean, 1e-6, None))
        features = np.concatenate([mean, mx, mn, std])  # [256]
        scales = np.array([1.0, scale_all[r], iscale_all[r]], dtype=np.float32)
        flat = (features[:, None] * scales[None, :]).reshape(-1)  # [768]
        res = flat @ wr.T + br
        nrm = np.linalg.norm(res)
        result[r] = res / max(nrm, 1e-12)
    return result


def kernel(text_embeddings, kgl2token, W_down, W_re, b_re, _trace=False):
    nc = _get_nc()
    in_maps, lengths, scale, iscale, invl = _prepare(
        text_embeddings, kgl2token, W_down, W_re, b_re
    )
    r = run_bass_kernel_spmd(nc, in_maps, core_ids=list(range(NCORES)), trace=_trace)
    outs = [r.results[c]["out"][:BSH] for c in range(NCORES)]
    result = np.concatenate(outs, axis=0).astype(np.float32)
    result = _patch_rows(
        result, text_embeddings, kgl2token, W_down, W_re, b_re, scale, iscale, invl
    )
    if _trace:
        return result, r
    return result
